# revision 18
# baseline (speedup 1.0000x reference)
"""Trainium2 Bass kernel for nn_CCSOFT (SO(3) cross-correlation via SOFT).

Math (validated vs reference):
  wig[l,m,k,n] factors as d[l,m,k]*d[l,k,n]  (rank-1 in (m,n) per (l,k)).
  Recover u[l,k,m]=d[l,m,k], v[l,k,n]=d[l,k,n] on host from wig, then fuse the
  lmkn contraction with the 3D inverse DFT (127 is prime -> DFT = matmul):
    E[x,m] = exp(+2j*pi*m*x/127)/127          (same matrix for all 3 axes)
    P[b,l,m,k] = F[b,l,m]*u[l,k,m]            F = f_re + i f_im
    A[b,l,x,k] = sum_m E[x,m] P[b,l,m,k]      (stage 1)
    Q[b,l,n,k] = G[b,l,n]*v[l,k,n]            G = conj(g)
    C[b,l,k,z] = sum_n E[z,n] Q[b,l,n,k]      (stage 2)
    S[b,k,x,z] = sum_l A[b,l,x,k] C[b,l,k,z]  (stage 3, contract (re/im,l)=128)
    out[b,x,y,z] = sum_k E[y,k] S[b,k,x,z]    (stage 4)

Data parallel over batch b: 32 batches -> 4 per core on 8 NeuronCores.
Layout rotations (to get l resp. k onto the partition axis) bounce through
DRAM scratch in bf16.  Complex stage-3 products use sign-on-A stacks:
  S_re = [Are; -Aim]^T [Cre; Cim],  S_im = [Aim; Are]^T [Cre; Cim]
so C needs a single un-negated stack.  Stage-1/2 matmuls, evacuations and
P/Q builds are pruned to the wigner band (|k-63| <= l).
All matmuls bf16 x bf16 -> fp32 PSUM.
"""

import sys

if "/opt/trn_rl_repo" not in sys.path:
    sys.path.insert(0, "/opt/trn_rl_repo")

import ml_dtypes
import numpy as np

import concourse.tile as tile
from concourse import bacc, mybir
from concourse.bass_utils import run_bass_kernel_spmd

B, L, M = 32, 64, 127
NCORES = 8
BC = B // NCORES          # batches per core
NJ = 16                   # stage1/2 chunks: 16 chunks x 4 l's
LJ = L // NJ              # 4
CH = LJ * M               # 508 columns per chunk
BF16 = mybir.dt.float16
F32 = mybir.dt.float32
NPBF = np.float16

_PROGS = {}  # reps -> compiled nc

# packed (l,k)-band layout for the d tables and P/Q: chunk j holds
# (l in [4j,4j+4)) x (k in [k0_j,k1_j)) contiguously at offset PK_OFF[j]
def _kwin(j):
    lmax = LJ * j + (LJ - 1)
    return max(0, 63 - lmax), min(M, 63 + lmax + 1)


PK_OFF = []
_o = 0
for _j in range(NJ):
    PK_OFF.append(_o)
    _k0, _k1 = _kwin(_j)
    _o += LJ * (_k1 - _k0)
PK_COLS = _o


def _factor_wig(wig):
    """wig (L,M,M,M) float32 -> u[l,k,m], v[l,k,n] with u*v^T == wig[l,:,k,:]."""
    R = np.ascontiguousarray(wig.transpose(0, 2, 1, 3))          # (l,k,m,n)
    Rf = R.reshape(L, M, M * M)
    idx = np.abs(Rf).argmax(-1)
    mstar, nstar = idx // M, idx % M
    s = np.take_along_axis(Rf, idx[..., None], -1)[..., 0]       # R[l,k,m*,n*]
    u = np.take_along_axis(R, nstar[..., None, None], 3)[..., 0]  # (l,k,m)
    v = np.take_along_axis(R, mstar[..., None, None], 2)[..., 0, :]  # (l,k,n)
    safe = np.abs(s) > 0
    v = np.where(safe[..., None], v / np.where(safe, s, 1)[..., None], 0.0)
    u = np.where(safe[..., None], u, 0.0)
    return u.astype(np.float32), v.astype(np.float32)


def _build_program(reps=1):
    nc = bacc.Bacc("TRN2", target_bir_lowering=False, debug=False,
                   num_devices=NCORES)

    # ---- external inputs (per core) ----
    # d-factor stacks: dstkM[m, l, k] = u[l,k,m]; dstkT[n, l, k] = v[l,k,n]
    dstkM = nc.dram_tensor("dstkM", [M, PK_COLS], BF16, kind="ExternalInput").ap()
    dstkT = nc.dram_tensor("dstkT", [M, PK_COLS], BF16, kind="ExternalInput").ap()
    # DFT matrices (symmetric): ex_re[m,x]=cos(2pi m x/127)/127, ex_im=sin/127
    exre_d = nc.dram_tensor("exre", [M, M], BF16, kind="ExternalInput").ap()
    exim_d = nc.dram_tensor("exim", [M, M], BF16, kind="ExternalInput").ap()
    eximn_d = nc.dram_tensor("eximn", [M, M], BF16, kind="ExternalInput").ap()
    # f/g tiles: [m, b, l] with b = per-core batch index; gimn = -g_im (conj)
    fre_d = nc.dram_tensor("fre", [M, BC, L], BF16, kind="ExternalInput").ap()
    fim_d = nc.dram_tensor("fim", [M, BC, L], BF16, kind="ExternalInput").ap()
    gre_d = nc.dram_tensor("gre", [M, BC, L], BF16, kind="ExternalInput").ap()
    gimn_d = nc.dram_tensor("gimn", [M, BC, L], BF16, kind="ExternalInput").ap()

    # ---- external outputs ----
    outre = nc.dram_tensor("outre", [BC, M, M, M], BF16, kind="ExternalOutput").ap()  # [b,y,x,z]
    outim = nc.dram_tensor("outim", [BC, M, M, M], BF16, kind="ExternalOutput").ap()  # [b,y,x,z]

    # ---- DRAM scratch (bounce buffers, bf16) ----
    # A: [c(re,im,imneg), l, x, k]; C: [c(re,im), l, z, k]; S: [k, c, x, z]
    Adram_t = nc.dram_tensor("Adram", [BC, 3, L, M, M], BF16).ap()
    Cdram_t = nc.dram_tensor("Cdram", [BC, 2, L, M, M], BF16).ap()
    Sdram_t = nc.dram_tensor("Sdram", [BC, M, 2, M, M], BF16).ap()

    from contextlib import ExitStack
    with tile.TileContext(nc) as tc, ExitStack() as ctx:
        cpool = ctx.enter_context(tc.tile_pool(name="consts", bufs=1))
        pqpool = ctx.enter_context(tc.tile_pool(name="pq", bufs=2))
        astkpool = ctx.enter_context(tc.tile_pool(name="astk", bufs=2))
        pool2 = ctx.enter_context(tc.tile_pool(name="cstk_sk", bufs=2))
        scr1 = ctx.enter_context(tc.tile_pool(name="scr1", bufs=6))
        scr3 = ctx.enter_context(tc.tile_pool(name="scr3", bufs=4))
        ps12 = ctx.enter_context(tc.tile_pool(name="ps12", bufs=4, space="PSUM"))
        ps34 = ctx.enter_context(tc.tile_pool(name="ps34", bufs=2, space="PSUM"))
        scr4 = ctx.enter_context(tc.tile_pool(name="scr4", bufs=3))
        skp = ctx.enter_context(tc.tile_pool(name="skp", bufs=6))

        # constants into SBUF
        dM = cpool.tile([M, PK_COLS], BF16, tag="dM")
        nc.sync.dma_start(dM[:], dstkM)
        dT = cpool.tile([M, PK_COLS], BF16, tag="dT")
        nc.scalar.dma_start(dT[:], dstkT)
        exre = cpool.tile([M, M], BF16, tag="exre")
        nc.gpsimd.dma_start(exre[:], exre_d)
        exim = cpool.tile([M, M], BF16, tag="exim")
        nc.gpsimd.dma_start(exim[:], exim_d)
        eximn = cpool.tile([M, M], BF16, tag="eximn")
        nc.gpsimd.dma_start(eximn[:], eximn_d)
        fgt = {}
        for nm, dr in (("fre", fre_d), ("fim", fim_d), ("gre", gre_d),
                       ("gimn", gimn_d)):
            t = cpool.tile([M, BC * L], BF16, tag=nm)
            nc.gpsimd.dma_start(t[:], dr.rearrange("m b l -> m (b l)"))
            fgt[nm] = t

        # one-time zero fill of the A/C bounce buffers: per-rep writes only
        # touch the wigner band, the rest must read back as zeros
        zt = cpool.tile([128, CH], BF16, tag="zt")
        nc.vector.memset(zt[:], 0.0)
        zeng = (nc.sync, nc.scalar, nc.gpsimd)
        zi = 0
        for flat, nrow in ((Adram_t.rearrange("b c l x k -> (b c l) (x k)"),
                            BC * 3 * L),
                           (Cdram_t.rearrange("b c l z k -> (b c l) (z k)"),
                            BC * 2 * L)):
            for g in range(nrow // 128):
                for c0 in range(0, M * M, CH):
                    w = min(CH, M * M - c0)
                    zeng[zi % 3].dma_start(
                        flat[g * 128:(g + 1) * 128, c0:c0 + w], zt[:, 0:w])
                    zi += 1

        for rep in range(reps):
            for b in range(BC):
                Adram = Adram_t[b]
                Cdram = Cdram_t[b]
                Sdram = Sdram_t[b]
                # ============ stage 2: Q build + C = E @ Q ============
                Q = pqpool.tile([M, 2 * PK_COLS], BF16, tag="pq")
                csc = {}
                for j in range(NJ):
                    k0, k1 = _kwin(j)
                    W = k1 - k0
                    po, pw = PK_OFF[j], LJ * (k1 - k0)
                    for ci, nm in enumerate(("gre", "gimn")):
                        gb = fgt[nm][:, b * L + j * LJ:b * L + (j + 1) * LJ]
                        nc.vector.tensor_tensor(
                            out=Q[:, ci * PK_COLS + po:ci * PK_COLS + po + pw]
                                .rearrange("n (l k) -> n l k", l=LJ),
                            in0=dT[:, po:po + pw].rearrange(
                                "n (l k) -> n l k", l=LJ),
                            in1=gb.broadcast_to((M, LJ, W)),
                            op=mybir.AluOpType.mult)
                    rre = Q[:, po:po + pw]
                    rim = Q[:, PK_COLS + po:PK_COLS + po + pw]
                    pc_re = ps12.tile([M, 512], F32, tag="ps")
                    nc.tensor.matmul(pc_re[:, 0:LJ * W], exre[:], rre, start=True, stop=False)
                    nc.tensor.matmul(pc_re[:, 0:LJ * W], eximn[:], rim, start=False, stop=True)
                    pc_im = ps12.tile([M, 512], F32, tag="ps")
                    nc.tensor.matmul(pc_im[:, 0:LJ * W], exim[:], rre, start=True, stop=False)
                    nc.tensor.matmul(pc_im[:, 0:LJ * W], exre[:], rim, start=False, stop=True)
                    for ci, ps in ((0, pc_re), (1, pc_im)):
                        cw = scr1.tile([M, CH], BF16, tag="scr1",
                                       name=f"csc{ci}")
                        half = cw[:, 0:LJ * W].rearrange(
                            "z (l k) -> z l k", l=LJ)
                        src = ps[:, 0:LJ * W].rearrange("z (l k) -> z l k", l=LJ)
                        if ci == 0:
                            nc.vector.tensor_copy(half, src)
                        else:
                            nc.scalar.mul(half, src, 1.0)
                        nc.scalar.dma_start(
                            Cdram[ci, j * LJ:(j + 1) * LJ, :, k0:k1].rearrange(
                                "l z k -> z l k"),
                            half)

                # C stack [C_re; C_im] (prefetched during stage 1)
                Cstk = pool2.tile([2 * L, M * M], BF16, tag="cstk")
                nc.gpsimd.dma_start(
                    Cstk[0:L].rearrange("l (z k) -> l z k", z=M), Cdram[0])
                nc.gpsimd.dma_start(
                    Cstk[L:2 * L].rearrange("l (z k) -> l z k", z=M), Cdram[1])

                # ============ stage 1: P build + A = E @ P ============
                P = pqpool.tile([M, 2 * PK_COLS], BF16, tag="pq")
                asc = {}
                for j in range(NJ):
                    k0, k1 = _kwin(j)
                    W = k1 - k0
                    po, pw = PK_OFF[j], LJ * (k1 - k0)
                    for ci, nm in enumerate(("fre", "fim")):
                        fb = fgt[nm][:, b * L + j * LJ:b * L + (j + 1) * LJ]
                        nc.vector.tensor_tensor(
                            out=P[:, ci * PK_COLS + po:ci * PK_COLS + po + pw]
                                .rearrange("m (l k) -> m l k", l=LJ),
                            in0=dM[:, po:po + pw].rearrange(
                                "m (l k) -> m l k", l=LJ),
                            in1=fb.broadcast_to((M, LJ, W)),
                            op=mybir.AluOpType.mult)
                    rre = P[:, po:po + pw]
                    rim = P[:, PK_COLS + po:PK_COLS + po + pw]
                    pa_re = ps12.tile([M, 512], F32, tag="ps")
                    nc.tensor.matmul(pa_re[:, 0:LJ * W], exre[:], rre, start=True, stop=False)
                    nc.tensor.matmul(pa_re[:, 0:LJ * W], eximn[:], rim, start=False, stop=True)
                    pa_im = ps12.tile([M, 512], F32, tag="ps")
                    nc.tensor.matmul(pa_im[:, 0:LJ * W], exim[:], rre, start=True, stop=False)
                    nc.tensor.matmul(pa_im[:, 0:LJ * W], exre[:], rim, start=False, stop=True)
                    # A comps: 0 = re, 1 = im, 2 = -im
                    for ci, ps, scl in ((0, pa_re, 1.0), (1, pa_im, 1.0),
                                        (2, pa_im, -1.0)):
                        aw = scr1.tile([M, CH], BF16, tag="scr1",
                                       name=f"asc{ci}")
                        half = aw[:, 0:LJ * W].rearrange(
                            "x (l k) -> x l k", l=LJ)
                        src = ps[:, 0:LJ * W].rearrange("x (l k) -> x l k", l=LJ)
                        if ci == 0:
                            nc.vector.tensor_copy(half, src)
                        else:
                            nc.scalar.mul(half, src, scl)
                        nc.sync.dma_start(
                            Adram[ci, j * LJ:(j + 1) * LJ, :, k0:k1].rearrange(
                                "l x k -> x l k"),
                            half)

                # A stacks: A1 = [Are; -Aim], A2 = [Aim; Are]
                Astk1 = astkpool.tile([2 * L, M * M], BF16, tag="astk")
                nc.scalar.dma_start(
                    Astk1[0:L].rearrange("l (x k) -> l x k", x=M), Adram[0])
                nc.scalar.dma_start(
                    Astk1[L:2 * L].rearrange("l (x k) -> l x k", x=M), Adram[2])
                Astk2 = astkpool.tile([2 * L, M * M], BF16, tag="astk")
                nc.gpsimd.dma_start(
                    Astk2[0:L].rearrange("l (x k) -> l x k", x=M), Adram[1])
                nc.gpsimd.dma_start(
                    Astk2[L:2 * L].rearrange("l (x k) -> l x k", x=M), Adram[0])

                # ============ stage 3: S[b,k] = sum_(c,l) A~ C~ ============
                A13 = Astk1[:].rearrange("p (x k) -> p x k", x=M)
                A23 = Astk2[:].rearrange("p (x k) -> p x k", x=M)
                CC3 = Cstk[:].rearrange("p (z k) -> p z k", z=M)
                s4 = Sdram.rearrange("k c x z -> x k c z")
                NG = (M + 1) // 2                                    # 64 k-groups
                for kg in range(NG):
                    kn = min(2, M - kg * 2)
                    psS = ps34.tile([M, 1024], F32, tag="ps")        # 2 banks
                    for t in range(kn):
                        k = kg * 2 + t
                        nc.tensor.matmul(psS[:, t * 512:t * 512 + M],
                                         A13[:, :, k], CC3[:, :, k],
                                         start=True, stop=True)
                        nc.tensor.matmul(psS[:, t * 512 + 256:t * 512 + 256 + M],
                                         A23[:, :, k], CC3[:, :, k],
                                         start=True, stop=True)
                    if kg % 2 == 0:
                        ssc = scr3.tile([M, 8 * M], BF16, tag="scr3")
                        ssc_k0 = kg * 2
                    pview = psS[:].rearrange("x (t c u) -> x t c u", t=2, c=2)
                    sv = ssc[:].rearrange("x (t c z) -> x t c z", t=4, c=2)
                    toff = (kg % 2) * 2
                    nc.vector.tensor_copy(sv[:, toff:toff + kn, :, 0:M],
                                          pview[:, 0:kn, :, 0:M])
                    if kg % 2 == 1 or kg == NG - 1:
                        ktot = kg * 2 + kn - ssc_k0
                        eng = nc.sync if (kg // 2) % 2 == 0 else nc.gpsimd
                        eng.dma_start(
                            s4[:, ssc_k0:ssc_k0 + ktot],
                            sv[:, 0:ktot, :, 0:M])

                # ============ stage 4: out[b] = E @ S ============
                ore = outre[b]   # [y, x, z] layout
                oim = outim[b]
                skeng = (nc.gpsimd, nc.sync, nc.scalar)
                for xg in range(32):                              # groups of 4 x's
                    xn = min(4, M - xg * 4)
                    cw = xn * M
                    x0 = xg * 4
                    c0t = skp.tile([M, 512], BF16, tag="sk", name="sk0")
                    skeng[xg % 3].dma_start(
                        c0t[:, 0:cw].rearrange("k (x z) -> k x z", x=xn),
                        Sdram[:, 0, x0:x0 + xn])
                    c1t = skp.tile([M, 512], BF16, tag="sk", name="sk1")
                    skeng[(xg + 1) % 3].dma_start(
                        c1t[:, 0:cw].rearrange("k (x z) -> k x z", x=xn),
                        Sdram[:, 1, x0:x0 + xn])
                    c0 = c0t[:, 0:cw]
                    c1 = c1t[:, 0:cw]
                    if xg % 2 == 0:
                        sore = scr4.tile([M, 8 * M], BF16, tag="scr4")
                        soim = scr4.tile([M, 8 * M], BF16, tag="scr4")
                        so_x0 = xg * 4
                    off = (xg % 2) * CH
                    po = ps34.tile([M, 1024], F32, tag="ps")
                    nc.tensor.matmul(po[:, 0:cw], exre[:], c0, start=True, stop=False)
                    nc.tensor.matmul(po[:, 0:cw], eximn[:], c1, start=False, stop=True)
                    nc.vector.tensor_copy(sore[:, off:off + cw], po[:, 0:cw])
                    po = ps34.tile([M, 1024], F32, tag="ps")
                    nc.tensor.matmul(po[:, 0:cw], exim[:], c0, start=True, stop=False)
                    nc.tensor.matmul(po[:, 0:cw], exre[:], c1, start=False, stop=True)
                    nc.scalar.mul(soim[:, off:off + cw], po[:, 0:cw], 1.0)
                    if xg % 2 == 1:
                        xtot = xg * 4 + xn - so_x0
                        eng = nc.sync if (xg // 2) % 2 == 0 else nc.scalar
                        eng.dma_start(
                            ore[:, so_x0:so_x0 + xtot],
                            sore[:, 0:off + cw].rearrange(
                                "y (x z) -> y x z", x=xtot))
                        eng.dma_start(
                            oim[:, so_x0:so_x0 + xtot],
                            soim[:, 0:off + cw].rearrange(
                                "y (x z) -> y x z", x=xtot))

    nc.compile()
    return nc


def _get_program(reps=1):
    if reps not in _PROGS:
        _PROGS[reps] = _build_program(reps)
    return _PROGS[reps]


def _pack_lk(t_mlk):
    """(m, l, k) float -> [m, PK_COLS] banded-packed bf16."""
    out = np.zeros((M, PK_COLS), dtype=np.float32)
    for j in range(NJ):
        k0, k1 = _kwin(j)
        blk = t_mlk[:, j * LJ:(j + 1) * LJ, k0:k1]       # (m, LJ, W)
        out[:, PK_OFF[j]:PK_OFF[j] + LJ * (k1 - k0)] = blk.reshape(M, -1)
    return out.astype(NPBF)


def _make_inmaps(f_re, f_im, g_re, g_im, wig):
    u, v = _factor_wig(np.asarray(wig, dtype=np.float32))
    # dstkM[m, packed(l,k)] = u[l,k,m]; dstkT[n, packed(l,k)] = v[l,k,n]
    dstkM = _pack_lk(u.transpose(2, 0, 1))
    dstkT = _pack_lk(v.transpose(2, 0, 1))
    ang = 2.0 * np.pi * np.outer(np.arange(M), np.arange(M)) / M
    exre = (np.cos(ang) / M).astype(NPBF)
    exim = (np.sin(ang) / M).astype(NPBF)
    eximn = (-np.sin(ang) / M).astype(NPBF)

    def fgt(x, sl, neg=False):
        t = np.asarray(x, dtype=np.float32)[sl]            # (BC, L, M)
        t = t.transpose(2, 0, 1)                           # (M, BC, L)
        if neg:
            t = -t
        return np.ascontiguousarray(t).astype(NPBF)

    in_maps = []
    for c in range(NCORES):
        sl = slice(c * BC, (c + 1) * BC)
        in_maps.append({
            "dstkM": dstkM, "dstkT": dstkT,
            "exre": exre, "exim": exim, "eximn": eximn,
            "fre": fgt(f_re, sl), "fim": fgt(f_im, sl),
            "gre": fgt(g_re, sl), "gimn": fgt(g_im, sl, neg=True),
        })
    return in_maps


def _assemble_output(results):
    out = np.empty((B, M, M, M), dtype=np.complex64)
    for c in range(NCORES):
        r = results[c]
        # device wrote [b, y, x, z]; reference order is [b, x, y, z]
        out[c * BC:(c + 1) * BC] = (
            r["outre"].astype(np.float32).transpose(0, 2, 1, 3)
            + 1j * r["outim"].astype(np.float32).transpose(0, 2, 1, 3))
    return out


def kernel(f_re, f_im, g_re, g_im, wig):
    nc = _get_program()
    in_maps = _make_inmaps(f_re, f_im, g_re, g_im, wig)
    res = run_bass_kernel_spmd(nc, in_maps, list(range(NCORES)))
    return _assemble_output(res.results)
